# revision 8
# baseline (speedup 1.0000x reference)
"""MoE routing kernel for Trainium2 (8 NeuronCores, expert-parallel).

Problem: top-2-of-8 expert MLP with squared-ReLU, d_model=1024, d_ff=1024,
N=8192 tokens. Strategy: the router (softmax + top-2, ~0.2% of FLOPs) runs
on host in float64; tokens are dispatched per expert on host (gather +
sqrt(combine-weight) scaling — relu(sqrt(w)*z)^2 == w*relu(z)^2, so the
combine weight folds into the input and the device kernel is a plain
2-layer MLP). Core e computes expert e over its (padded) token batch with
float32r matmuls; host scatter-adds the per-expert outputs.
"""

import sys

if "/opt/trn_rl_repo" not in sys.path:
    sys.path.insert(0, "/opt/trn_rl_repo")

import numpy as np

import bass_rust
import concourse.bass as bass
import concourse.tile as tile
from concourse import mybir
from concourse.bass_utils import run_bass_kernel_spmd
from concourse.vector_clock import ScopedClock

NUM_EXPERTS = 8
TOP_K = 2
D_MODEL = 1024
D_FF = 1024
N_CORES = 8

# ---------------------------------------------------------------------------
# Compat: this container's walrus rejects instructions carrying more than one
# sem wait ("Too many sync wait commands"). Replace the TileContext final
# drain with single-wait SP nops, and post-process the module so every
# instruction carries at most one (monotonic) wait.
# ---------------------------------------------------------------------------


def _patched_drain_and_barrier(self, tick_clock, wait_clock):
    probe = self.nc.sync.nop(nofuse=True)
    wait_clock.add_sem_waits(probe.ins, ScopedClock({None: tick_clock.global_clock}))
    si = probe.ins.sync_info
    waits = list(si.on_wait) if si is not None else []
    updates = list(si.on_update) if si is not None else []
    if len(waits) > 1:
        probe.ins.sync_info = bass_rust.SyncInfo(on_wait=[waits[0]], on_update=updates)
        for w in waits[1:]:
            extra = self.nc.sync.nop(nofuse=True)
            extra.ins.sync_info = bass_rust.SyncInfo(on_wait=[w], on_update=[])
    self.nc.sync.drain()
    self.nc.all_engine_barrier()
    assert self.sems is not None
    popped = self.nc._tile_sem_poison_stack.pop()
    assert popped is self._sem_poison
    self.nc.clear_and_free_semaphores(list(self.sems.allocated().values()))
    self.nc.all_engine_barrier()


tile.TileContext._drain_and_barrier = _patched_drain_and_barrier


def split_excess_waits(nc, limit=1):
    for fn in nc.m.functions:
        for bb in fn.blocks:
            il = bb.instructions
            i = 0
            while i < len(il):
                inst = il[i]
                si = inst.sync_info
                if si is not None and len(si.on_wait) > limit:
                    waits = list(si.on_wait)
                    movable = [w for w in waits if "ge" in (w.wait_mode or "")]
                    pinned = [w for w in waits if w not in movable]
                    keep_n = max(0, limit - len(pinned))
                    if keep_n:
                        keep = pinned + movable[len(movable) - keep_n :]
                        extra = movable[: len(movable) - keep_n]
                    else:
                        keep, extra = pinned, movable
                    if not extra:
                        i += 1
                        continue
                    nops = []
                    for w in extra:
                        nop = mybir.InstNoOp(
                            name=nc.get_next_instruction_name(), ins=[], outs=[]
                        )
                        nop.engine = inst.engine
                        nop.sync_info = bass_rust.SyncInfo(on_wait=[w], on_update=[])
                        nops.append(nop)
                    inst.sync_info = bass_rust.SyncInfo(
                        on_wait=keep, on_update=list(si.on_update)
                    )
                    for j, nop in enumerate(nops):
                        il.insert(i + j, nop)
                    i += len(nops)
                i += 1


# ---------------------------------------------------------------------------
# Device program: per-core 2-layer MLP with squared ReLU, feature-major.
#   xT  [D_MODEL, cap] f32r   (tokens pre-scaled by sqrt(combine weight))
#   w1  [D_MODEL, D_FF] f32r, w2 [D_FF, D_MODEL] f32r
#   yT  [D_MODEL, cap] f32
# ---------------------------------------------------------------------------

F32R = mybir.dt.float32r
F32 = mybir.dt.float32


def _token_blocks(cap):
    # Small first block so the first PSUM group's dependencies (one weight
    # chunk + one small x block) land quickly; 512-wide steady state.
    blocks = []
    t = 0
    while t < cap:
        if t == 0 and cap % 512 == 256:
            tb = 256
        else:
            tb = 512 if cap - t >= 512 else cap - t
        blocks.append((t, tb))
        t += tb
    return blocks


def build_program(cap):
    KC = D_MODEL // 128
    FT = D_FF // 128
    DT = D_MODEL // 128

    nc = bass.Bass("TRN2", target_bir_lowering=False, debug=False, num_devices=N_CORES)
    xT = nc.declare_dram_parameter("xT", [D_MODEL, cap], F32R, isOutput=False)
    w1 = nc.declare_dram_parameter("w1", [D_MODEL, D_FF], F32R, isOutput=False)
    w2 = nc.declare_dram_parameter("w2", [D_FF, D_MODEL], F32R, isOutput=False)
    yT = nc.declare_dram_parameter("yT", [D_MODEL, cap], F32, isOutput=True)

    # w1/w2 arrive host-prepacked in consumption order: row block ft of w1
    # holds [p, (kc c)] = W1[kc*128+p, ft*128+c], so each output-tile group
    # depends on exactly one contiguous 512 KiB DMA.
    xT_r = xT.rearrange("(kc p) t -> kc p t", p=128)
    w1_r = w1.rearrange("(ft p) x -> ft p x", p=128)
    w2_r = w2.rearrange("(dt p) x -> dt p x", p=128)
    yT_r = yT.rearrange("(dt p) t -> dt p t", p=128)

    with tile.TileContext(nc) as tc:
        with (
            tc.tile_pool(name="wpool", bufs=1) as wpool,
            tc.tile_pool(name="xpool", bufs=3) as xpool,
            tc.tile_pool(name="mpool", bufs=2) as mpool,
            tc.tile_pool(name="tpool", bufs=3) as tpool,
            tc.tile_pool(name="opool", bufs=3) as opool,
            tc.tile_pool(name="psum", bufs=2, space="PSUM") as psum_pool,
        ):
            w1_sb = wpool.tile([128, FT * D_MODEL], F32R, tag="w1")
            w2_sb = wpool.tile([128, DT * D_FF], F32R, tag="w2")
            for ft in range(FT):
                nc.sync.dma_start(
                    w1_sb[:, ft * D_MODEL : (ft + 1) * D_MODEL], w1_r[ft]
                )
            for dt_ in range(DT):
                nc.sync.dma_start(
                    w2_sb[:, dt_ * D_FF : (dt_ + 1) * D_FF], w2_r[dt_]
                )

            for t0, tb in _token_blocks(cap):
                x_sb = xpool.tile([128, KC * tb], F32R, tag="x")
                for kc in range(KC):
                    nc.sync.dma_start(
                        x_sb[:, kc * tb : (kc + 1) * tb],
                        xT_r[kc, :, t0 : t0 + tb],
                    )
                mid_sb = mpool.tile([128, FT * tb], F32R, tag="mid")
                for ft in range(FT):
                    ps = psum_pool.tile([128, tb], F32, tag="ps")
                    for kc in range(KC):
                        nc.tensor.matmul(
                            ps[:],
                            w1_sb[
                                :, ft * D_MODEL + kc * 128 : ft * D_MODEL + kc * 128 + 128
                            ],
                            x_sb[:, kc * tb : (kc + 1) * tb],
                            start=(kc == 0),
                            stop=(kc == KC - 1),
                        )
                    tmp = tpool.tile([128, tb], F32, tag="tmp")
                    nc.scalar.activation(
                        tmp[:], ps[:], mybir.ActivationFunctionType.Relu
                    )
                    nc.vector.tensor_mul(
                        mid_sb[:, ft * tb : (ft + 1) * tb], tmp[:], tmp[:]
                    )
                for dt_ in range(DT):
                    ps2 = psum_pool.tile([128, tb], F32, tag="ps2")
                    for fc in range(FT):
                        nc.tensor.matmul(
                            ps2[:],
                            w2_sb[
                                :, dt_ * D_FF + fc * 128 : dt_ * D_FF + fc * 128 + 128
                            ],
                            mid_sb[:, fc * tb : (fc + 1) * tb],
                            start=(fc == 0),
                            stop=(fc == FT - 1),
                        )
                    o_sb = opool.tile([128, tb], F32, tag="o")
                    nc.scalar.copy(o_sb[:], ps2[:])
                    nc.sync.dma_start(yT_r[dt_, :, t0 : t0 + tb], o_sb[:])

    split_excess_waits(nc, limit=1)
    return nc


_PROGRAM_CACHE = {}


def _get_program(cap):
    if cap not in _PROGRAM_CACHE:
        _PROGRAM_CACHE[cap] = build_program(cap)
    return _PROGRAM_CACHE[cap]


# ---------------------------------------------------------------------------
# Host side: routing, dispatch, combine.
# ---------------------------------------------------------------------------


def _prep_weight(w):
    """[K, M] -> row block m of the result holds [p, (kc c)] = w[kc*128+p,
    m*128+c], so the on-device [128,128] lhsT tiles for output-tile m are one
    contiguous row block."""
    k, m = w.shape
    return np.ascontiguousarray(
        w.reshape(k // 128, 128, m // 128, 128)
        .transpose(2, 1, 0, 3)
        .reshape(m // 128 * 128, k // 128 * 128),
        dtype=np.float32,
    )


def kernel(x, Wr, W1, W2, _trace=False):
    x = np.asarray(x)
    Wr = np.asarray(Wr)
    W1 = np.asarray(W1)
    W2 = np.asarray(W2)
    B, T, C = x.shape
    N = B * T
    xf = np.ascontiguousarray(x.reshape(N, C), dtype=np.float32)

    # Router in float64 (matches jax f32 top_k selections; verified).
    logits = xf.astype(np.float64) @ Wr.astype(np.float64)
    logits -= logits.max(axis=-1, keepdims=True)
    p = np.exp(logits)
    p /= p.sum(axis=-1, keepdims=True)
    idx = np.argsort(-p, axis=-1, kind="stable")[:, :TOP_K]  # [N, K]
    wts = np.take_along_axis(p, idx, axis=-1)  # [N, K]

    # Dispatch list sorted by expert.
    flat_e = idx.ravel()
    order = np.argsort(flat_e, kind="stable")
    tok_of_pair = np.repeat(np.arange(N), TOP_K)[order]
    w_of_pair = wts.ravel()[order]
    counts = np.bincount(flat_e, minlength=NUM_EXPERTS)
    starts = np.concatenate([[0], np.cumsum(counts)[:-1]])

    cap = int(max(256, -(-int(counts.max()) // 256) * 256))

    in_maps = []
    toks_per_e = []
    for e in range(NUM_EXPERTS):
        s, c = int(starts[e]), int(counts[e])
        toks = tok_of_pair[s : s + c]
        toks_per_e.append(toks)
        ws = w_of_pair[s : s + c].astype(np.float32)
        xg = xf[toks] * np.sqrt(ws)[:, None]  # [c, C]
        xTe = np.zeros((C, cap), np.float32)
        xTe[:, :c] = xg.T
        in_maps.append(
            {
                "xT": xTe,
                "w1": _prep_weight(W1[e]),
                "w2": _prep_weight(W2[e]),
            }
        )

    nc = _get_program(cap)
    res = run_bass_kernel_spmd(
        nc, in_maps, core_ids=list(range(N_CORES)), trace=_trace
    )

    out = np.zeros((N, C), np.float32)
    for e in range(NUM_EXPERTS):
        c = int(counts[e])
        if c:
            out[toks_per_e[e]] += res.results[e]["yT"][:, :c].T
    if _trace:
        kernel._last_exec_time_ns = res.exec_time_ns
    return out.reshape(B, T, C)
